# revision 1
# baseline (speedup 1.0000x reference)
"""Trainium2 Bass kernel for nn_Cross_Attention (B=16, C=256, H=W=96).

reference:
    q = Z1.reshape(B, C, N); k = Zr.reshape(B, C, N)         # N = H*W
    energy    = q @ k^T                                       # [B, C, C]
    attention = softmax(rowmax(energy) - energy, axis=-1)
    out       = attention @ k                                 # [B, C, N]
    return beta * out + Zr

Strategy: data-parallel over batch, 2 batches per NeuronCore on 8 cores.
Uploads per core: q^T in bf16 (host pre-packed [P, NT, C] partition-major so
the N-contraction matmul streams straight from DRAM) and Zr in f32.  k is
derived on-chip: kb = bf16(Zr) (ScalarE downcast) feeds the second matmul
directly and is transposed tile-by-tile on the TensorE (transpose-mode
matmul) into k^T tiles for the energy matmul — so k crosses HBM once.
softmax(max - e) == exp(min - e) / sum(exp(min - e)) row-wise: only a
row-min is needed, exp args are always <= 0 (no overflow), the sum is >= 1
(no div-by-0).  beta and 1/sum are folded into the attention weights before
the second matmul, so the final blend is a single add with the f32-resident
Zr (bitwise-exact output when beta == 0).
"""

from contextlib import ExitStack

import ml_dtypes
import numpy as np

import concourse.bass as bass
import concourse.tile as tile
from concourse import bacc, mybir
from concourse.bass_utils import run_bass_kernel_spmd
from concourse.masks import make_identity

B, C, H, W = 16, 256, 96, 96
N = H * W                    # 9216
P = 128
NCORES = 8
BL = B // NCORES             # 2 batches per core
CT = C // P                  # 2 c-tiles of 128
NT = N // P                  # 72 contraction tiles for energy
TCH = 18                     # qt tiles per DMA chunk
NCH = NT // TCH              # 4 chunks (last one split per c-tile)
TQT = (NCH - 1) * TCH        # 54 t-tiles in the interleaved qt tensor
NH = N // 2                  # 4608: kb slice width (half a c-tile row)
NQ = N // 4                  # 2304: zr tile width (quarter c-tile row)
TPH = NH // P                # 36 n-tiles per h-half
OW = 384                     # mm2 psum chunk width (6 per zr quarter)
WPH = NH // OW               # 12 psum chunks per h-half

F32 = mybir.dt.float32
BF16 = mybir.dt.bfloat16


def _build_program():
    nc = bacc.Bacc("TRN2", target_bir_lowering=False, debug=False,
                   num_devices=NCORES)

    qt_ext = nc.dram_tensor("qt", [BL, P, TQT, C], BF16, kind="ExternalInput")
    qtt_ext = nc.dram_tensor("qtt", [BL, CT, P, TCH, P], BF16,
                             kind="ExternalInput")
    zr_ext = nc.dram_tensor("zr", [BL, C, N], F32, kind="ExternalInput")
    beta_ext = nc.dram_tensor("beta", [1], F32, kind="ExternalInput")
    out_ext = nc.dram_tensor("out", [BL, C, N], F32, kind="ExternalOutput")

    with tile.TileContext(nc) as tc, ExitStack() as ctx:
        qtp = ctx.enter_context(tc.tile_pool(name="qtp", bufs=3))
        zrp = ctx.enter_context(tc.tile_pool(name="zrp", bufs=12))
        kbp = ctx.enter_context(tc.tile_pool(name="kbp", bufs=6))
        kttp = ctx.enter_context(tc.tile_pool(name="kttp", bufs=4))
        expp = ctx.enter_context(tc.tile_pool(name="expp", bufs=2))
        attp = ctx.enter_context(tc.tile_pool(name="attp", bufs=2))
        atTp = ctx.enter_context(tc.tile_pool(name="atTp", bufs=2))
        statp = ctx.enter_context(tc.tile_pool(name="statp", bufs=8))
        singles = ctx.enter_context(tc.tile_pool(name="singles", bufs=1))
        engp = ctx.enter_context(tc.tile_pool(name="engp", bufs=2, space="PSUM"))
        trp = ctx.enter_context(tc.tile_pool(name="trp", bufs=4, space="PSUM"))
        outp = ctx.enter_context(tc.tile_pool(name="outp", bufs=2, space="PSUM"))

        ident = singles.tile([P, P], BF16)
        make_identity(nc, ident)
        beta_sb = singles.tile([P, 1], F32)
        nc.gpsimd.dma_start(out=beta_sb, in_=beta_ext.ap().to_broadcast((P, 1)))

        deferred_stores = []
        for b in range(BL):
            # ---- interleaved load/compute pipeline: chunk i of the
            # energy matmul consumes zr quarter i (via the kb downcast and
            # PE transposes) and qt chunk i, so the sync-ring order
            # [zr(.,qi), qt_i] feeds compute just-in-time ----
            zr_tiles = {}
            kb = {}
            eng = [engp.tile([P, C], F32, name="eng") for _ in range(CT)]
            for i in range(NCH - 1):
                h, qq = divmod(i, 2)
                for cj in range(CT):
                    zt = zrp.tile([P, NQ], F32)
                    nc.sync.dma_start(
                        out=zt,
                        in_=zr_ext[b, cj * P:(cj + 1) * P, i * NQ:(i + 1) * NQ],
                    )
                    zr_tiles[cj, i] = zt
                for cj in range(CT):
                    if qq == 0:
                        kb[cj, h] = kbp.tile([P, NH], BF16, name="kb_t")
                    nc.scalar.copy(out=kb[cj, h][:, qq * NQ:(qq + 1) * NQ],
                                   in_=zr_tiles[cj, i])
                qt_t = qtp.tile([P, TCH, C], BF16)
                nc.sync.dma_start(out=qt_t, in_=qt_ext[b, :, i * TCH:(i + 1) * TCH, :])
                # transpose+copy producers, then this chunk's matmuls
                ktts = []
                for tg in range(TCH // 4):
                    tr4 = trp.tile([P, 4, CT, P], BF16, name="tr4")
                    for tq in range(4):
                        t = i * TCH + tg * 4 + tq
                        th = t - h * TPH
                        for dj in range(CT):
                            nc.tensor.transpose(tr4[:, tq, dj, :],
                                                kb[dj, h][:, th * P:(th + 1) * P],
                                                ident)
                    ktt4 = kttp.tile([P, 4, CT * P], BF16, name="ktt4")
                    nc.scalar.copy(out=ktt4, in_=tr4)
                    ktts.extend(ktt4[:, tq, :] for tq in range(4))
                for tl in range(TCH // 4 * 4, TCH):
                    t = i * TCH + tl
                    th = t - h * TPH
                    tr2 = trp.tile([P, 4, CT, P], BF16, name="tr2", tag="tr4")
                    for dj in range(CT):
                        nc.tensor.transpose(tr2[:, 0, dj, :],
                                            kb[dj, h][:, th * P:(th + 1) * P],
                                            ident)
                    ktt1 = kttp.tile([P, 4, CT * P], BF16, name="ktt1", tag="ktt4")
                    nc.scalar.copy(out=ktt1[:, 0, :], in_=tr2[:, 0, :, :])
                    ktts.append(ktt1[:, 0, :])
                for tl in range(TCH):
                    t = i * TCH + tl
                    for ci in range(CT):
                        nc.tensor.matmul(
                            eng[ci],
                            lhsT=qt_t[:, tl, ci * P:(ci + 1) * P],
                            rhs=ktts[tl],
                            start=(t == 0),
                            stop=False,
                        )

            # ---- final chunk, split per c-tile: eng[0] closes a full qt
            # sub-load earlier than eng[1], so its softmax / mm2 / stores
            # overlap the ci=1 stream ----
            i = NCH - 1
            h, qq = divmod(i, 2)
            for cj in range(CT):
                zt = zrp.tile([P, NQ], F32)
                nc.sync.dma_start(
                    out=zt,
                    in_=zr_ext[b, cj * P:(cj + 1) * P, i * NQ:(i + 1) * NQ],
                )
                zr_tiles[cj, i] = zt
            for cj in range(CT):
                nc.scalar.copy(out=kb[cj, h][:, qq * NQ:(qq + 1) * NQ],
                               in_=zr_tiles[cj, i])
            ktts = []
            for tg in range(TCH // 4):
                tr4 = trp.tile([P, 4, CT, P], BF16, name="tr4")
                for tq in range(4):
                    t = i * TCH + tg * 4 + tq
                    th = t - h * TPH
                    for dj in range(CT):
                        nc.tensor.transpose(tr4[:, tq, dj, :],
                                            kb[dj, h][:, th * P:(th + 1) * P],
                                            ident)
                ktt4 = kttp.tile([P, 4, CT * P], BF16, name="ktt4")
                nc.scalar.copy(out=ktt4, in_=tr4)
                ktts.extend(ktt4[:, tq, :] for tq in range(4))
            for tl in range(TCH // 4 * 4, TCH):
                t = i * TCH + tl
                th = t - h * TPH
                tr2 = trp.tile([P, 4, CT, P], BF16, name="tr2", tag="tr4")
                for dj in range(CT):
                    nc.tensor.transpose(tr2[:, 0, dj, :],
                                        kb[dj, h][:, th * P:(th + 1) * P],
                                        ident)
                ktt1 = kttp.tile([P, 4, CT * P], BF16, name="ktt1", tag="ktt4")
                nc.scalar.copy(out=ktt1[:, 0, :], in_=tr2[:, 0, :, :])
                ktts.append(ktt1[:, 0, :])
            for ci in range(CT):
                qtt_t = qtp.tile([P, TCH, P], BF16, name="qtt_t", tag="qt_t")
                nc.sync.dma_start(out=qtt_t, in_=qtt_ext[b, ci])
                for tl in range(TCH):
                    t = i * TCH + tl
                    nc.tensor.matmul(
                        eng[ci],
                        lhsT=qtt_t[:, tl, :],
                        rhs=ktts[tl],
                        start=False,
                        stop=(t == NT - 1),
                    )

            if b == BL - 1:
                for dst_ap, src_t in deferred_stores:
                    nc.sync.dma_start(out=dst_ap, in_=src_t)
                deferred_stores = []

            # ---- softmax(max-e) = exp(min-e)/sum; fold beta/sum in.
            # Per-ci attnT tiles keep mm2(ci=0) independent of softmax(1) ----
            attnT = []
            for ci in range(CT):
                mn = statp.tile([P, 1], F32)
                nc.vector.tensor_reduce(out=mn, in_=eng[ci],
                                        axis=mybir.AxisListType.X,
                                        op=mybir.AluOpType.min)
                ex = expp.tile([P, C], F32)
                sm = statp.tile([P, 1], F32)
                nc.scalar.activation(out=ex, in_=eng[ci],
                                     func=mybir.ActivationFunctionType.Exp,
                                     bias=mn, scale=-1.0, accum_out=sm)
                rc = statp.tile([P, 1], F32)
                nc.vector.reciprocal(out=rc, in_=sm)
                rb = statp.tile([P, 1], F32)
                nc.vector.tensor_mul(out=rb, in0=rc, in1=beta_sb)
                at = attp.tile([P, C], BF16)
                nc.vector.tensor_scalar_mul(out=at, in0=ex, scalar1=rb)
                trA = trp.tile([P, CT, P], BF16, name="trA", tag="tr4")
                for dj in range(CT):
                    nc.tensor.transpose(trA[:, dj, :],
                                        at[:, dj * P:(dj + 1) * P], ident)
                atT = atTp.tile([P, CT, P], BF16, name="atT")
                nc.vector.tensor_copy(out=atT, in_=trA)
                attnT.append(atT)

            # ---- out = attn @ k, blended in place into zr, streamed out ----
            # h-outer so the n-low half's stores launch while later work
            # streams; each 4608-wide slice is stored in two 2304-wide pieces
            for ci in range(CT):
                for h in range(2):
                    for qq in range(2):
                        q = h * 2 + qq
                        zt = zr_tiles[ci, q]
                        for wq in range(WPH // 2):
                            w = qq * (WPH // 2) + wq
                            ps = outp.tile([P, OW], F32)
                            for dj in range(CT):
                                nc.tensor.matmul(
                                    ps,
                                    lhsT=attnT[ci][:, dj, :],
                                    rhs=kb[dj, h][:, w * OW:(w + 1) * OW],
                                    start=(dj == 0),
                                    stop=(dj == CT - 1),
                                )
                            nc.vector.tensor_add(
                                out=zt[:, wq * OW:(wq + 1) * OW],
                                in0=ps,
                                in1=zt[:, wq * OW:(wq + 1) * OW])
                        if b == BL - 1:
                            # final batch: 768-wide store pieces so the
                            # store stream starts ~2 blends earlier
                            for hp in range(3):
                                w_ = NQ // 3
                                nc.sync.dma_start(
                                    out=out_ext[b, ci * P:(ci + 1) * P,
                                                q * NQ + hp * w_:
                                                q * NQ + (hp + 1) * w_],
                                    in_=zt[:, hp * w_:(hp + 1) * w_],
                                )
                        elif ci == 1:
                            # deferred into the final batch's tail: frees
                            # mid-kernel DMA bandwidth for its loads and
                            # fills the pre-store dependency gap
                            deferred_stores.append(
                                (out_ext[b, ci * P:(ci + 1) * P,
                                         q * NQ:(q + 1) * NQ], zt))
                        else:
                            nc.gpsimd.dma_start(
                                out=out_ext[b, ci * P:(ci + 1) * P,
                                            q * NQ:(q + 1) * NQ],
                                in_=zt,
                            )

    nc.compile()
    return nc


_NC_CACHE = None


def _get_program():
    global _NC_CACHE
    if _NC_CACHE is None:
        _NC_CACHE = _build_program()
    return _NC_CACHE


def pack_qt(Z1):
    # bf16 q^T, partition-major: full[b, p, t, c] = q[b, c, t*128+p];
    # t < TQT interleaved-ci ("qt"), the last chunk split per ci ("qtt")
    x = Z1.reshape(B, C, NT, P).astype(ml_dtypes.bfloat16)
    full = x.transpose(0, 3, 2, 1)
    qta = np.ascontiguousarray(full[:, :, :TQT, :])
    qtb = np.ascontiguousarray(
        full[:, :, TQT:, :].reshape(B, P, TCH, CT, P).transpose(0, 3, 1, 2, 4))
    return qta, qtb


def kernel(Z1, Zr, beta):
    Z1 = np.asarray(Z1, dtype=np.float32)
    Zr = np.asarray(Zr, dtype=np.float32)
    beta = np.asarray(beta, dtype=np.float32).reshape(1)

    qta, qtb = pack_qt(Z1)
    zr = np.ascontiguousarray(Zr.reshape(B, C, N))

    in_maps = []
    for i in range(NCORES):
        s = slice(i * BL, (i + 1) * BL)
        in_maps.append({"qt": qta[s], "qtt": qtb[s], "zr": zr[s],
                        "beta": beta})

    nc = _get_program()
    res = run_bass_kernel_spmd(nc, in_maps, list(range(NCORES)))
    out = np.concatenate([r["out"] for r in res.results], axis=0)
    return out.reshape(B, C, H, W)



# revision 3
# speedup vs baseline: 1.8071x; 1.8071x over previous
"""Trainium2 Bass kernel for nn_Cross_Attention (B=16, C=256, H=W=96).

reference:
    q = Z1.reshape(B, C, N); k = Zr.reshape(B, C, N)         # N = H*W
    energy    = q @ k^T                                       # [B, C, C]
    attention = softmax(rowmax(energy) - energy, axis=-1)
    out       = attention @ k                                 # [B, C, N]
    return beta * out + Zr

Strategy: data-parallel over batch, 2 batches per NeuronCore on 8 cores.
All device I/O is fp8e4m3 and all matmuls run in fp8 with DoubleRow perf
mode (one PE instruction contracts a 256-deep pair of k-tiles), which cuts
both the HBM traffic and the PE time ~2x vs a bf16 formulation:
  - q^T is host-packed fp8 [P, 36, 2, C] (contraction-pair-major) so the
    energy matmul streams straight from DRAM with no on-chip transposes.
  - k  is the fp8 downcast of Zr, loaded once [C, N]; the energy matmul's
    k^T pair-tiles are derived on-chip with PE transpose-mode matmuls
    (fp8 transposes must write PSUM with element step 2 - walrus rule -
    so the psum tr tiles carry a trailing pad dim and the psum->SBUF
    repack copies read strided).
  - softmax(max - e) == exp(min - e) / sum(exp(min - e)) row-wise: only a
    row-min is needed, exp args are <= 0 (no overflow), sum >= 1.
  - beta and 1/sum are folded into the attention weights BEFORE the second
    matmul, so the device emits delta := beta * (attn @ k) in fp8 and the
    host adds the f32 residual:  out = Zr + delta.  When beta == 0 the
    folded weights are exactly zero, delta is exactly zero, and the
    returned output is bitwise Zr.
The psum->SBUF repack copies (k^T tiles and the delta downcast) are load
balanced across ScalarE / VectorE / GpSimdE so no single engine exceeds
the ~39us/core DMA roofline (14.2 MB of fp8 traffic at 360 GB/s).
"""

from contextlib import ExitStack

import ml_dtypes
import numpy as np

import concourse.bass as bass
import concourse.tile as tile
from concourse import bacc, mybir
from concourse.bass_utils import run_bass_kernel_spmd
from concourse.masks import make_identity

B, C, H, W = 16, 256, 96, 96
N = H * W                    # 9216
P = 128
NCORES = 8
BL = B // NCORES             # 2 batches per core
CT = C // P                  # 2 c-tiles of 128
NT = N // P                  # 72 contraction tiles
NPAIR = NT // 2              # 36 DoubleRow contraction pairs
QCH = 12                     # qt pairs per DMA chunk -> 3 chunks
GP = 2                       # pairs per transpose/repack group
NG = NPAIR // GP             # 18 groups per batch
KCC = 4                      # kb column chunks per c-tile row
KCW = N // KCC               # 2304 cols per kb chunk
OW = 512                     # mm2 psum chunk width
OPC = 2                      # mm2 psum chunks per outp tile -> copies of 1024
DCW = OPC * OW               # 1024: delta repack width
NDC = N // DCW               # 9 delta repacks per c-tile row
SCW = 3 * DCW                # 3072: store width (3 repacks per store)

F32 = mybir.dt.float32
FP8 = mybir.dt.float8e4
NP_FP8 = ml_dtypes.float8_e4m3
DR = mybir.MatmulPerfMode.DoubleRow


class _CopyBalancer:
    """Round-robin psum->SBUF repack copies across ACT/DVE/Pool by
    projected busy-ns so no engine becomes the bottleneck."""

    def __init__(self, nc):
        # (issue fn, ns per element, fixed ns per instruction).  GpSimd is
        # excluded: it cannot access PSUM (BIR verifier rule).
        self.engines = [
            [nc.scalar.copy, 0.833, 250.0, 0.0],
            [lambda out, in_: nc.vector.tensor_copy(out=out, in_=in_), 1.042, 215.0, 0.0],
        ]

    def charge(self, idx, ns):
        self.engines[idx][3] += ns

    def copy(self, out, in_, free):
        best = min(self.engines, key=lambda e: e[3] + free * e[1] + e[2])
        best[3] += free * best[1] + best[2]
        if best is self.engines[0]:
            best[0](out=out, in_=in_)
        else:
            best[0](out, in_)


def _build_program():
    nc = bacc.Bacc("TRN2", target_bir_lowering=False, debug=False,
                   num_devices=NCORES)

    qt_ext = nc.dram_tensor("qt", [BL, P, NPAIR, 2, C], FP8,
                            kind="ExternalInput")
    zr_ext = nc.dram_tensor("zr", [BL, C, N], FP8, kind="ExternalInput")
    beta_ext = nc.dram_tensor("beta", [1], F32, kind="ExternalInput")
    out_ext = nc.dram_tensor("out", [BL, C, N], FP8, kind="ExternalOutput")

    with tile.TileContext(nc) as tc, ExitStack() as ctx:
        kbp = ctx.enter_context(tc.tile_pool(name="kbp", bufs=2))
        qtp = ctx.enter_context(tc.tile_pool(name="qtp", bufs=6))
        kttp = ctx.enter_context(tc.tile_pool(name="kttp", bufs=4))
        expp = ctx.enter_context(tc.tile_pool(name="expp", bufs=2))
        attp = ctx.enter_context(tc.tile_pool(name="attp", bufs=2))
        atTp = ctx.enter_context(tc.tile_pool(name="atTp", bufs=2))
        deltap = ctx.enter_context(tc.tile_pool(name="deltap", bufs=3))
        statp = ctx.enter_context(tc.tile_pool(name="statp", bufs=8))
        singles = ctx.enter_context(tc.tile_pool(name="singles", bufs=1))
        engp = ctx.enter_context(tc.tile_pool(name="engp", bufs=2, space="PSUM"))
        trp = ctx.enter_context(tc.tile_pool(name="trp", bufs=2, space="PSUM"))
        outp = ctx.enter_context(tc.tile_pool(name="outp", bufs=2, space="PSUM"))

        cb = _CopyBalancer(nc)

        ident = singles.tile([P, P], FP8)
        make_identity(nc, ident)
        beta_sb = singles.tile([P, 1], F32)
        nc.gpsimd.dma_start(out=beta_sb, in_=beta_ext.ap().to_broadcast((P, 1)))

        # ---- all loads up front on the sync queue: the DMA device order is
        # loads(b0), loads(b1), stores(b0), stores(b1), so stores never block
        # a load and the 360 GB/s stream stays saturated ----
        kb = []
        qt = []
        for b in range(BL):
            kb.append(kbp.tile([P, CT, N], FP8, name="kb"))
            qt.append([qtp.tile([P, QCH, 2, C], FP8, name="qt_t")
                       for _ in range(NPAIR // QCH)])
            for cc in range(KCC):
                for cj in range(CT):
                    nc.sync.dma_start(
                        out=kb[b][:, cj, cc * KCW:(cc + 1) * KCW],
                        in_=zr_ext[b, cj * P:(cj + 1) * P,
                                   cc * KCW:(cc + 1) * KCW],
                    )
                if cc < NPAIR // QCH:
                    nc.sync.dma_start(
                        out=qt[b][cc],
                        in_=qt_ext[b, :, cc * QCH:(cc + 1) * QCH, :, :])

        for b in range(BL):
            # ---- energy = q @ k^T via DoubleRow fp8 matmuls; k^T pair
            # tiles derived on-chip (PE transpose, step-2 psum, repack).
            # One-group emission lag so the PE never waits on a repack ----
            eng = [engp.tile([P, C], F32, name="eng") for _ in range(CT)]
            ktts = [None] * NG

            def emit_tr(g, b=b):
                trt = trp.tile([P, GP, 2, CT, P, 2], FP8, name="trt")
                for pr in range(GP):
                    for j in range(2):
                        t = (GP * g + pr) * 2 + j
                        for dj in range(CT):
                            nc.tensor.transpose(
                                trt[:, pr, j, dj, :, 0],
                                kb[b][:, dj, t * P:(t + 1) * P],
                                ident)
                ktt = kttp.tile([P, GP, 2, CT * P], FP8, name="ktt")
                cb.copy(ktt, trt[:, :, :, :, :, 0], GP * 2 * CT * P)
                ktts[g] = ktt

            def emit_mm(g, b=b):
                for pr in range(GP):
                    t2 = GP * g + pr
                    for ci in range(CT):
                        nc.tensor.matmul(
                            eng[ci],
                            lhsT=qt[b][t2 // QCH][:, t2 % QCH, :,
                                                  ci * P:(ci + 1) * P],
                            rhs=ktts[g][:, pr, :, :],
                            start=(t2 == 0),
                            stop=(t2 == NPAIR - 1),
                            perf_mode=DR,
                        )

            emit_tr(0)
            for g in range(1, NG):
                emit_tr(g)
                emit_mm(g - 1)
            emit_mm(NG - 1)

            # ---- softmax(max-e) = exp(min-e)/sum with beta/sum folded in;
            # attention emitted fp8 and pair-transposed for the DR mm2 ----
            atT = []
            for ci in range(CT):
                mn = statp.tile([P, 1], F32)
                nc.vector.tensor_reduce(out=mn, in_=eng[ci],
                                        axis=mybir.AxisListType.X,
                                        op=mybir.AluOpType.min)
                ex = expp.tile([P, C], F32)
                sm = statp.tile([P, 1], F32)
                nc.scalar.activation(out=ex, in_=eng[ci],
                                     func=mybir.ActivationFunctionType.Exp,
                                     bias=mn, scale=-1.0, accum_out=sm)
                rc = statp.tile([P, 1], F32)
                nc.vector.reciprocal(out=rc, in_=sm)
                rb = statp.tile([P, 1], F32)
                nc.vector.tensor_mul(out=rb, in0=rc, in1=beta_sb)
                at = attp.tile([P, C], FP8)
                nc.vector.tensor_scalar_mul(out=at, in0=ex, scalar1=rb)
                atr = trp.tile([P, GP, 2, CT, P, 2], FP8, name="atr",
                               tag="trt")
                for dj in range(CT):
                    nc.tensor.transpose(atr[:, 0, 0, dj, :, 0],
                                        at[:, dj * P:(dj + 1) * P], ident)
                att = atTp.tile([P, CT, P], FP8, name="atT")
                cb.copy(att, atr[:, 0, 0, :, :, 0], CT * P)
                atT.append(att)
            cb.charge(0, 1600)   # exp x2 on ACT
            cb.charge(1, 3000)   # softmax smalls on DVE

            # ---- delta = attn_scaled @ k: one DR matmul per 512-wide psum
            # chunk (full 256-deep contraction), repack to fp8, stream out ----
            for ci in range(CT):
                delta = deltap.tile([P, N], FP8, name="delta")
                for w2 in range(NDC):
                    ps = outp.tile([P, OPC, OW], F32, name="ps")
                    for q in range(OPC):
                        w0 = w2 * DCW + q * OW
                        nc.tensor.matmul(
                            ps[:, q, :],
                            lhsT=atT[ci],
                            rhs=kb[b][:, :, w0:w0 + OW],
                            start=True, stop=True,
                            perf_mode=DR,
                        )
                    cb.copy(delta[:, w2 * DCW:(w2 + 1) * DCW], ps, DCW)
                for s in range(N // SCW):
                    nc.sync.dma_start(
                        out=out_ext[b, ci * P:(ci + 1) * P,
                                    s * SCW:(s + 1) * SCW],
                        in_=delta[:, s * SCW:(s + 1) * SCW],
                    )

    nc.compile()
    return nc


_NC_CACHE = None


def _get_program():
    global _NC_CACHE
    if _NC_CACHE is None:
        _NC_CACHE = _build_program()
    return _NC_CACHE


def pack_qt(Z1):
    # fp8 q^T, contraction-pair-major: qt[b, p, t2, j, c] = q[b, c, n] with
    # n = (2*t2 + j)*128 + p, matching the DoubleRow lhsT pair layout
    x = Z1.reshape(B, C, NT, P).astype(NP_FP8)
    return np.ascontiguousarray(x.transpose(0, 3, 2, 1)).reshape(
        B, P, NPAIR, 2, C)


def kernel(Z1, Zr, beta):
    Z1 = np.asarray(Z1, dtype=np.float32)
    Zr = np.asarray(Zr, dtype=np.float32)
    beta = np.asarray(beta, dtype=np.float32).reshape(1)

    qta = pack_qt(Z1)
    zrk = np.ascontiguousarray(Zr.reshape(B, C, N)).astype(NP_FP8)

    in_maps = []
    for i in range(NCORES):
        s = slice(i * BL, (i + 1) * BL)
        in_maps.append({"qt": qta[s], "zr": zrk[s], "beta": beta})

    nc = _get_program()
    res = run_bass_kernel_spmd(nc, in_maps, list(range(NCORES)))
    delta = np.concatenate(
        [np.asarray(r["out"]).astype(np.float32) for r in res.results], axis=0)
    return (Zr.reshape(B, C, N) + delta).reshape(B, C, H, W)


# revision 8
# speedup vs baseline: 1.9386x; 1.0728x over previous
"""Trainium2 Bass kernel for nn_Cross_Attention (B=16, C=256, H=W=96).

reference:
    q = Z1.reshape(B, C, N); k = Zr.reshape(B, C, N)         # N = H*W
    energy    = q @ k^T                                       # [B, C, C]
    attention = softmax(rowmax(energy) - energy, axis=-1)
    out       = attention @ k                                 # [B, C, N]
    return beta * out + Zr

Strategy: data-parallel over batch, 2 batches per NeuronCore on 8 cores.
All device I/O is fp8e4m3 and all matmuls run in fp8 with DoubleRow perf
mode (one PE instruction contracts a 256-deep pair of k-tiles), which cuts
both the HBM traffic and the PE time ~2x vs a bf16 formulation:
  - q^T is host-packed fp8 [P, 36, 2, C] (contraction-pair-major) so the
    energy matmul streams straight from DRAM with no on-chip transposes.
  - k  is the fp8 downcast of Zr, loaded once [C, N]; the energy matmul's
    k^T pair-tiles are derived on-chip with PE transpose-mode matmuls
    (fp8 transposes must write PSUM with element step 2 - walrus rule -
    so the psum tr tiles carry a trailing pad dim and the psum->SBUF
    repack copies read strided).
  - softmax(max - e) == exp(min - e) / sum(exp(min - e)) row-wise: only a
    row-min is needed, exp args are <= 0 (no overflow), sum >= 1.
  - beta and 1/sum are folded into the attention weights BEFORE the second
    matmul, so the device emits delta := beta * (attn @ k) in fp8 and the
    host adds the f32 residual:  out = Zr + delta.  When beta == 0 the
    folded weights are exactly zero, delta is exactly zero, and the
    returned output is bitwise Zr.
The psum->SBUF repack copies (k^T tiles and the delta downcast) are load
balanced across ScalarE / VectorE / GpSimdE so no single engine exceeds
the ~39us/core DMA roofline (14.2 MB of fp8 traffic at 360 GB/s).
"""

from contextlib import ExitStack

import ml_dtypes
import numpy as np

import concourse.bass as bass
import concourse.tile as tile
from concourse import bacc, mybir
from concourse.bass_utils import run_bass_kernel_spmd
from concourse.masks import make_identity

B, C, H, W = 16, 256, 96, 96
N = H * W                    # 9216
P = 128
NCORES = 8
BL = B // NCORES             # 2 batches per core
CT = C // P                  # 2 c-tiles of 128
NT = N // P                  # 72 contraction tiles
NPAIR = NT // 2              # 36 DoubleRow contraction pairs
QCH = 12                     # qt pairs per DMA chunk -> 3 chunks
GP = 2                       # pairs per transpose/repack group
NG = NPAIR // GP             # 18 groups per batch
KCC = 4                      # kb column chunks per c-tile row
KCW = N // KCC               # 2304 cols per kb chunk
OW = 512                     # mm2 psum chunk width
OPC = 2                      # mm2 psum chunks per outp tile -> copies of 1024
DCW = OPC * OW               # 1024: delta repack width
NDC = N // DCW               # 9 delta repacks per c-tile row
SCW = 3 * DCW                # 3072: store width (3 repacks per store)

F32 = mybir.dt.float32
FP8 = mybir.dt.float8e4
NP_FP8 = ml_dtypes.float8_e4m3
DR = mybir.MatmulPerfMode.DoubleRow


class _CopyBalancer:
    """Round-robin psum->SBUF repack copies across ACT/DVE/Pool by
    projected busy-ns so no engine becomes the bottleneck."""

    def __init__(self, nc):
        # (issue fn, ns per element, fixed ns per instruction).  GpSimd is
        # excluded: it cannot access PSUM (BIR verifier rule).
        self.engines = [
            [nc.scalar.copy, 0.833, 250.0, 0.0],
            [lambda out, in_: nc.vector.tensor_copy(out=out, in_=in_), 1.042, 215.0, 0.0],
        ]

    def charge(self, idx, ns):
        self.engines[idx][3] += ns

    def copy(self, out, in_, free):
        best = min(self.engines, key=lambda e: e[3] + free * e[1] + e[2])
        best[3] += free * best[1] + best[2]
        if best is self.engines[0]:
            best[0](out=out, in_=in_)
        else:
            best[0](out, in_)


def _build_program():
    nc = bacc.Bacc("TRN2", target_bir_lowering=False, debug=False,
                   num_devices=NCORES)

    qt_ext = nc.dram_tensor("qt", [BL, P, NPAIR, 2, C], FP8,
                            kind="ExternalInput")
    zr_ext = nc.dram_tensor("zr", [BL, C, N], FP8, kind="ExternalInput")
    beta_ext = nc.dram_tensor("beta", [1], F32, kind="ExternalInput")
    out_ext = nc.dram_tensor("out", [BL, C, N], FP8, kind="ExternalOutput")

    with tile.TileContext(nc) as tc, ExitStack() as ctx:
        kbp = ctx.enter_context(tc.tile_pool(name="kbp", bufs=2))
        qtp = ctx.enter_context(tc.tile_pool(name="qtp", bufs=6))
        kttp = ctx.enter_context(tc.tile_pool(name="kttp", bufs=8))
        expp = ctx.enter_context(tc.tile_pool(name="expp", bufs=2))
        attp = ctx.enter_context(tc.tile_pool(name="attp", bufs=2))
        atTp = ctx.enter_context(tc.tile_pool(name="atTp", bufs=2))
        deltap = ctx.enter_context(tc.tile_pool(name="deltap", bufs=4))
        statp = ctx.enter_context(tc.tile_pool(name="statp", bufs=8))
        singles = ctx.enter_context(tc.tile_pool(name="singles", bufs=1))
        engp = ctx.enter_context(tc.tile_pool(name="engp", bufs=1, space="PSUM"))
        trp = ctx.enter_context(tc.tile_pool(name="trp", bufs=3, space="PSUM"))
        outp = ctx.enter_context(tc.tile_pool(name="outp", bufs=2, space="PSUM"))

        cb = _CopyBalancer(nc)

        ident = singles.tile([P, P], FP8)
        make_identity(nc, ident)
        beta_sb = singles.tile([P, 1], F32)
        nc.gpsimd.dma_start(out=beta_sb, in_=beta_ext.ap().to_broadcast((P, 1)))

        # ---- emission helpers.  fp8 PE transposes must write PSUM with
        # element step 2 (walrus rule); the two n-tiles of a DoubleRow pair
        # interleave byte-wise into the same psum region so no space is
        # wasted, and the repack copy un-interleaves via a permuted AP ----
        def emit_tr(kb_b, g, ktts):
            trt = trp.tile([P, GP, 2, CT, P, 2], FP8, name="trt")
            for pr in range(GP):
                for j in range(2):
                    t = (GP * g + pr) * 2 + j
                    for dj in range(CT):
                        nc.tensor.transpose(
                            trt[:, pr, j, dj, :, 0],
                            kb_b[:, dj, t * P:(t + 1) * P],
                            ident)
            ktt = kttp.tile([P, GP, 2, CT * P], FP8, name="ktt")
            cb.copy(ktt, trt[:, :, :, :, :, 0], GP * 2 * CT * P)
            ktts[g] = ktt

        def emit_emm(qt_b, g, eng, ktts):
            for pr in range(GP):
                t2 = GP * g + pr
                for ci in range(CT):
                    nc.tensor.matmul(
                        eng[ci],
                        lhsT=qt_b[t2 // QCH][:, t2 % QCH, :,
                                             ci * P:(ci + 1) * P],
                        rhs=ktts[g][:, pr, :, :],
                        start=(t2 == 0),
                        stop=(t2 == NPAIR - 1),
                        perf_mode=DR,
                    )

        def emit_softmax(eng):
            # softmax(max-e) = exp(min-e)/sum with beta/sum folded into the
            # fp8 attention weights; pair-transposed for the DR mm2
            atT = []
            for ci in range(CT):
                mn = statp.tile([P, 1], F32)
                nc.vector.tensor_reduce(out=mn, in_=eng[ci],
                                        axis=mybir.AxisListType.X,
                                        op=mybir.AluOpType.min)
                ex = expp.tile([P, C], F32)
                sm = statp.tile([P, 1], F32)
                nc.scalar.activation(out=ex, in_=eng[ci],
                                     func=mybir.ActivationFunctionType.Exp,
                                     bias=mn, scale=-1.0, accum_out=sm)
                rc = statp.tile([P, 1], F32)
                nc.vector.reciprocal(out=rc, in_=sm)
                rb = statp.tile([P, 1], F32)
                nc.vector.tensor_mul(out=rb, in0=rc, in1=beta_sb)
                at = attp.tile([P, C], FP8)
                nc.vector.tensor_scalar_mul(out=at, in0=ex, scalar1=rb)
                atr = trp.tile([P, GP, 2, CT, P, 2], FP8, name="atr",
                               tag="trt")
                for dj in range(CT):
                    nc.tensor.transpose(atr[:, 0, 0, dj, :, 0],
                                        at[:, dj * P:(dj + 1) * P], ident)
                att = atTp.tile([P, CT, P], FP8, name="atT")
                cb.copy(att, atr[:, 0, 0, :, :, 0], CT * P)
                atT.append(att)
            cb.charge(0, 1600)   # exp x2 on ACT
            cb.charge(1, 3000)   # softmax smalls on DVE
            return atT

        def emit_mm2_chunk(b, kb_b, atT, deltas, ci, w2):
            # one 1024-wide slice of delta = attn_scaled @ k: two DR
            # matmuls (full 256-deep contraction each), fp8 repack, store
            ps = outp.tile([P, OPC, OW], F32, name="ps")
            for q in range(OPC):
                w0 = w2 * DCW + q * OW
                nc.tensor.matmul(
                    ps[:, q, :],
                    lhsT=atT[ci],
                    rhs=kb_b[:, :, w0:w0 + OW],
                    start=True, stop=True,
                    perf_mode=DR,
                )
            cb.copy(deltas[ci][:, w2 * DCW:(w2 + 1) * DCW], ps, DCW)
            nc.sync.dma_start(
                out=out_ext[b, ci * P:(ci + 1) * P,
                            w2 * DCW:(w2 + 1) * DCW],
                in_=deltas[ci][:, w2 * DCW:(w2 + 1) * DCW],
            )

        # ---- all loads up front on the sync queue: the DMA device order is
        # loads(b0), loads(b1), stores(b0), stores(b1), so stores never block
        # a load and the 360 GB/s stream stays saturated ----
        kb = []
        qt = []
        for b in range(BL):
            kb.append(kbp.tile([P, CT, N], FP8, name="kb"))
            qt.append([qtp.tile([P, QCH, 2, C], FP8, name="qt_t")
                       for _ in range(NPAIR // QCH)])
            for cc in range(KCC):
                for cj in range(CT):
                    nc.sync.dma_start(
                        out=kb[b][:, cj, cc * KCW:(cc + 1) * KCW],
                        in_=zr_ext[b, cj * P:(cj + 1) * P,
                                   cc * KCW:(cc + 1) * KCW],
                    )
                if cc < NPAIR // QCH:
                    nc.sync.dma_start(
                        out=qt[b][cc],
                        in_=qt_ext[b, :, cc * QCH:(cc + 1) * QCH, :, :])

        # ---- two-batch software pipeline.  Batch 0's energy streams behind
        # its loads; then batch 0's mm2 chunks interleave 1:1 with batch 1's
        # energy groups so the PE queue never head-of-line blocks and the
        # ACT/DVE repack streams stay packed; batch 1's mm2 is the tail ----
        engsl = [engp.tile([P, CT, C], F32, name="eng") for _ in range(BL)]
        eng = [[engsl[b][:, ci, :] for ci in range(CT)] for b in range(BL)]
        ktts = [[None] * NG for _ in range(BL)]
        deltas = [[deltap.tile([P, N], FP8, name="delta") for _ in range(CT)]
                  for _ in range(BL)]
        chunks = [(ci, w2) for ci in range(CT) for w2 in range(NDC)]

        emit_tr(kb[0], 0, ktts[0])
        for g in range(1, NG):
            emit_tr(kb[0], g, ktts[0])
            emit_emm(qt[0], g - 1, eng[0], ktts[0])
        emit_emm(qt[0], NG - 1, eng[0], ktts[0])
        atT0 = emit_softmax(eng[0])

        emit_tr(kb[1], 0, ktts[1])
        for i in range(NG):
            if i + 1 < NG:
                emit_tr(kb[1], i + 1, ktts[1])
            emit_mm2_chunk(0, kb[0], atT0, deltas[0], *chunks[i])
            emit_emm(qt[1], i, eng[1], ktts[1])
        atT1 = emit_softmax(eng[1])

        for ci, w2 in chunks:
            emit_mm2_chunk(1, kb[1], atT1, deltas[1], ci, w2)

    nc.compile()
    return nc


_NC_CACHE = None


def _get_program():
    global _NC_CACHE
    if _NC_CACHE is None:
        _NC_CACHE = _build_program()
    return _NC_CACHE


def pack_qt(Z1):
    # fp8 q^T, contraction-pair-major: qt[b, p, t2, j, c] = q[b, c, n] with
    # n = (2*t2 + j)*128 + p, matching the DoubleRow lhsT pair layout
    x = Z1.reshape(B, C, NT, P).astype(NP_FP8)
    return np.ascontiguousarray(x.transpose(0, 3, 2, 1)).reshape(
        B, P, NPAIR, 2, C)


def kernel(Z1, Zr, beta):
    Z1 = np.asarray(Z1, dtype=np.float32)
    Zr = np.asarray(Zr, dtype=np.float32)
    beta = np.asarray(beta, dtype=np.float32).reshape(1)

    qta = pack_qt(Z1)
    zrk = np.ascontiguousarray(Zr.reshape(B, C, N)).astype(NP_FP8)

    in_maps = []
    for i in range(NCORES):
        s = slice(i * BL, (i + 1) * BL)
        in_maps.append({"qt": qta[s], "zr": zrk[s], "beta": beta})

    nc = _get_program()
    res = run_bass_kernel_spmd(nc, in_maps, list(range(NCORES)))
    delta = np.concatenate(
        [np.asarray(r["out"]).astype(np.float32) for r in res.results], axis=0)
    return (Zr.reshape(B, C, N) + delta).reshape(B, C, H, W)


# revision 10
# speedup vs baseline: 1.9864x; 1.0247x over previous
"""Trainium2 Bass kernel for nn_Cross_Attention (B=16, C=256, H=W=96).

reference:
    q = Z1.reshape(B, C, N); k = Zr.reshape(B, C, N)         # N = H*W
    energy    = q @ k^T                                       # [B, C, C]
    attention = softmax(rowmax(energy) - energy, axis=-1)
    out       = attention @ k                                 # [B, C, N]
    return beta * out + Zr

Strategy: data-parallel over batch, 2 batches per NeuronCore on 8 cores.
All device I/O is fp8e4m3 and all matmuls run in fp8 with DoubleRow perf
mode (one PE instruction contracts a 256-deep pair of k-tiles), which cuts
both the HBM traffic and the PE time ~2x vs a bf16 formulation:
  - q^T is host-packed fp8 [P, 36, 2, C] (contraction-pair-major) so the
    energy matmul streams straight from DRAM with no on-chip transposes.
  - k  is the fp8 downcast of Zr, loaded once [C, N]; the energy matmul's
    k^T pair-tiles are derived on-chip with PE transpose-mode matmuls
    (fp8 transposes must write PSUM with element step 2 - walrus rule -
    so the psum tr tiles carry a trailing pad dim and the psum->SBUF
    repack copies read strided).
  - softmax(max - e) == exp(min - e) / sum(exp(min - e)) row-wise: only a
    row-min is needed, exp args are <= 0 (no overflow), sum >= 1.
  - beta and 1/sum are folded into the attention weights BEFORE the second
    matmul, so the device emits delta := beta * (attn @ k) in fp8 and the
    host adds the f32 residual:  out = Zr + delta.  When beta == 0 the
    folded weights are exactly zero, delta is exactly zero, and the
    returned output is bitwise Zr.
The psum->SBUF repack copies (k^T tiles and the delta downcast) are load
balanced across ScalarE / VectorE / GpSimdE so no single engine exceeds
the ~39us/core DMA roofline (14.2 MB of fp8 traffic at 360 GB/s).
"""

from contextlib import ExitStack

import ml_dtypes
import numpy as np

import concourse.bass as bass
import concourse.tile as tile
from concourse import bacc, mybir
from concourse.bass_utils import run_bass_kernel_spmd
from concourse.masks import make_identity

B, C, H, W = 16, 256, 96, 96
N = H * W                    # 9216
P = 128
NCORES = 8
BL = B // NCORES             # 2 batches per core
CT = C // P                  # 2 c-tiles of 128
NT = N // P                  # 72 contraction tiles
NPAIR = NT // 2              # 36 DoubleRow contraction pairs
QCH = 12                     # qt pairs per DMA chunk -> 3 chunks
GP = 2                       # pairs per transpose/repack group
NG = NPAIR // GP             # 18 groups per batch
KCC = 4                      # kb column chunks per c-tile row
KCW = N // KCC               # 2304 cols per kb chunk
OW = 512                     # mm2 psum chunk width
OPC = 2                      # mm2 psum chunks per outp tile -> copies of 1024
DCW = OPC * OW               # 1024: delta repack width
NDC = N // DCW               # 9 delta repacks per c-tile row
SCW = 3 * DCW                # 3072: store width (3 repacks per store)

F32 = mybir.dt.float32
U16 = mybir.dt.uint16
FP8 = mybir.dt.float8e4
NP_FP8 = ml_dtypes.float8_e4m3
DR = mybir.MatmulPerfMode.DoubleRow


class _CopyBalancer:
    """Round-robin psum->SBUF repack copies across ACT/DVE/Pool by
    projected busy-ns so no engine becomes the bottleneck."""

    def __init__(self, nc):
        # (issue fn, ns/elem plain, ns/elem 2x-capable, fixed ns).  GpSimd
        # is excluded: it cannot access PSUM (BIR verifier rule).  Only the
        # DVE has the 2x_1p fast path (2-byte packed operands).
        self.engines = [
            [nc.scalar.copy, 0.833, 0.833, 250.0, 0.0],
            [lambda out, in_: nc.vector.tensor_copy(out=out, in_=in_),
             1.042, 0.521, 215.0, 0.0],
        ]

    def charge(self, idx, ns):
        self.engines[idx][4] += ns

    def copy(self, out, in_, free, twox=False):
        r = 2 if twox else 1
        best = min(self.engines, key=lambda e: e[4] + free * e[r] + e[3])
        best[4] += free * best[r] + best[3]
        if best is self.engines[0]:
            best[0](out=out, in_=in_)
        else:
            best[0](out, in_)


def _build_program():
    nc = bacc.Bacc("TRN2", target_bir_lowering=False, debug=False,
                   num_devices=NCORES)

    qt_ext = nc.dram_tensor("qt", [BL, P, NPAIR, 2, C], FP8,
                            kind="ExternalInput")
    zr_ext = nc.dram_tensor("zr", [BL, C, N], FP8, kind="ExternalInput")
    beta_ext = nc.dram_tensor("beta", [1], F32, kind="ExternalInput")
    out_ext = nc.dram_tensor("out", [BL, C, N], FP8, kind="ExternalOutput")

    with tile.TileContext(nc) as tc, ExitStack() as ctx:
        kbp = ctx.enter_context(tc.tile_pool(name="kbp", bufs=2))
        qtp = ctx.enter_context(tc.tile_pool(name="qtp", bufs=6))
        kttp = ctx.enter_context(tc.tile_pool(name="kttp", bufs=8))
        expp = ctx.enter_context(tc.tile_pool(name="expp", bufs=2))
        attp = ctx.enter_context(tc.tile_pool(name="attp", bufs=2))
        atTp = ctx.enter_context(tc.tile_pool(name="atTp", bufs=2))
        deltap = ctx.enter_context(tc.tile_pool(name="deltap", bufs=4))
        statp = ctx.enter_context(tc.tile_pool(name="statp", bufs=8))
        singles = ctx.enter_context(tc.tile_pool(name="singles", bufs=1))
        engp = ctx.enter_context(tc.tile_pool(name="engp", bufs=1, space="PSUM"))
        trp = ctx.enter_context(tc.tile_pool(name="trp", bufs=3, space="PSUM"))
        outp = ctx.enter_context(tc.tile_pool(name="outp", bufs=2, space="PSUM"))

        cb = _CopyBalancer(nc)

        ident = singles.tile([P, P], FP8)
        make_identity(nc, ident)
        beta_sb = singles.tile([P, 1], F32)
        nc.gpsimd.dma_start(out=beta_sb, in_=beta_ext.ap().to_broadcast((P, 1)))

        # ---- emission helpers.  fp8 PE transposes must write PSUM with
        # element step 2 (walrus rule); the two n-tiles of a DoubleRow pair
        # interleave byte-wise into the same psum region so no space is
        # wasted, and the repack copy un-interleaves via a permuted AP ----
        def emit_tr(kb_b, g, ktts):
            trt = trp.tile([P, GP, 2, CT, P, 2], FP8, name="trt")
            for pr in range(GP):
                for j in range(2):
                    t = (GP * g + pr) * 2 + j
                    for dj in range(CT):
                        nc.tensor.transpose(
                            trt[:, pr, j, dj, :, 0],
                            kb_b[:, dj, t * P:(t + 1) * P],
                            ident)
            ktt = kttp.tile([P, GP, 2, CT * P], U16, name="ktt")
            cb.copy(ktt, trt.bitcast(U16), GP * 2 * CT * P, twox=True)
            ktts[g] = ktt.bitcast(FP8)

        def emit_emm(qt_b, g, eng, ktts):
            for pr in range(GP):
                t2 = GP * g + pr
                for ci in range(CT):
                    nc.tensor.matmul(
                        eng[ci],
                        lhsT=qt_b[t2 // QCH][:, t2 % QCH, :,
                                             ci * P:(ci + 1) * P],
                        rhs=ktts[g][:, pr, :, ::2],
                        start=(t2 == 0),
                        stop=(t2 == NPAIR - 1),
                        perf_mode=DR,
                    )

        def emit_softmax(eng):
            # softmax(max-e) = exp(min-e)/sum with beta/sum folded into the
            # fp8 attention weights; pair-transposed for the DR mm2
            atT = []
            for ci in range(CT):
                mn = statp.tile([P, 1], F32)
                nc.vector.tensor_reduce(out=mn, in_=eng[ci],
                                        axis=mybir.AxisListType.X,
                                        op=mybir.AluOpType.min)
                ex = expp.tile([P, C], F32)
                sm = statp.tile([P, 1], F32)
                nc.scalar.activation(out=ex, in_=eng[ci],
                                     func=mybir.ActivationFunctionType.Exp,
                                     bias=mn, scale=-1.0, accum_out=sm)
                rc = statp.tile([P, 1], F32)
                nc.vector.reciprocal(out=rc, in_=sm)
                rb = statp.tile([P, 1], F32)
                nc.vector.tensor_mul(out=rb, in0=rc, in1=beta_sb)
                at = attp.tile([P, C], FP8)
                nc.vector.tensor_scalar_mul(out=at, in0=ex, scalar1=rb)
                atr = trp.tile([P, GP, 2, CT, P, 2], FP8, name="atr",
                               tag="trt")
                for dj in range(CT):
                    nc.tensor.transpose(atr[:, 0, 0, dj, :, 0],
                                        at[:, dj * P:(dj + 1) * P], ident)
                att = atTp.tile([P, CT, P], FP8, name="atT")
                cb.copy(att, atr[:, 0, 0, :, :, 0], CT * P)
                atT.append(att)
            cb.charge(0, 1600)   # exp x2 on ACT
            cb.charge(1, 3000)   # softmax smalls on DVE
            return atT

        def emit_mm2_chunk(b, kb_b, atT, deltas, ci, off, width, borrow):
            # one slice of delta = attn_scaled @ k: DR matmuls with the full
            # 256-deep contraction per 512 of width, fp8 repack, store.  In
            # the batch-1 tail the transpose psum banks are dead, so every
            # other chunk borrows one (borrow=True) for a ~ring-5 pipeline
            if borrow:
                ps = trp.tile([P, width // OW, OW], F32, name="pst",
                              tag="trt")
            else:
                ps = outp.tile([P, width // OW, OW], F32, name="ps",
                               tag="ps")
            for q in range(width // OW):
                w0 = off + q * OW
                nc.tensor.matmul(
                    ps[:, q, :],
                    lhsT=atT[ci],
                    rhs=kb_b[:, :, w0:w0 + OW],
                    start=True, stop=True,
                    perf_mode=DR,
                )
            cb.copy(deltas[ci][:, off:off + width], ps, width)
            nc.sync.dma_start(
                out=out_ext[b, ci * P:(ci + 1) * P, off:off + width],
                in_=deltas[ci][:, off:off + width],
            )

        # ---- all loads up front on the sync queue: the DMA device order is
        # loads(b0), loads(b1), stores(b0), stores(b1), so stores never block
        # a load and the 360 GB/s stream stays saturated ----
        kb = []
        qt = []
        for b in range(BL):
            kb.append(kbp.tile([P, CT, N], FP8, name="kb"))
            qt.append([qtp.tile([P, QCH, 2, C], FP8, name="qt_t")
                       for _ in range(NPAIR // QCH)])
            for cc in range(KCC):
                for cj in range(CT):
                    nc.sync.dma_start(
                        out=kb[b][:, cj, cc * KCW:(cc + 1) * KCW],
                        in_=zr_ext[b, cj * P:(cj + 1) * P,
                                   cc * KCW:(cc + 1) * KCW],
                    )
            for cc in range(NPAIR // QCH):
                nc.sync.dma_start(
                    out=qt[b][cc],
                    in_=qt_ext[b, :, cc * QCH:(cc + 1) * QCH, :, :])

        # ---- two-batch software pipeline.  Batch 0's energy streams behind
        # its loads; then batch 0's mm2 chunks interleave 1:1 with batch 1's
        # energy groups so the PE queue never head-of-line blocks and the
        # ACT/DVE repack streams stay packed; batch 1's mm2 is the tail ----
        engsl = [engp.tile([P, CT, C], F32, name="eng") for _ in range(BL)]
        eng = [[engsl[b][:, ci, :] for ci in range(CT)] for b in range(BL)]
        ktts = [[None] * NG for _ in range(BL)]
        deltas = [[deltap.tile([P, N], FP8, name="delta") for _ in range(CT)]
                  for _ in range(BL)]
        chunks = [(ci, w2 * DCW, DCW) for ci in range(CT)
                  for w2 in range(NDC)]

        emit_tr(kb[0], 0, ktts[0])
        for g in range(1, NG):
            emit_tr(kb[0], g, ktts[0])
            emit_emm(qt[0], g - 1, eng[0], ktts[0])
        emit_emm(qt[0], NG - 1, eng[0], ktts[0])
        atT0 = emit_softmax(eng[0])

        emit_tr(kb[1], 0, ktts[1])
        for i in range(NG):
            if i + 1 < NG:
                emit_tr(kb[1], i + 1, ktts[1])
            emit_mm2_chunk(0, kb[0], atT0, deltas[0], *chunks[i], False)
            emit_emm(qt[1], i, eng[1], ktts[1])
        atT1 = emit_softmax(eng[1])

        # batch-1 tail: alternate 1024-wide outp chunks with 512-wide
        # borrowed-transpose-bank chunks for a deeper psum ring
        for ci in range(CT):
            for s in range(N // 1536):
                emit_mm2_chunk(1, kb[1], atT1, deltas[1], ci,
                               s * 1536, 1024, False)
                emit_mm2_chunk(1, kb[1], atT1, deltas[1], ci,
                               s * 1536 + 1024, 512, True)

    nc.compile()
    return nc


_NC_CACHE = None


def _get_program():
    global _NC_CACHE
    if _NC_CACHE is None:
        _NC_CACHE = _build_program()
    return _NC_CACHE


def pack_qt(Z1):
    # fp8 q^T, contraction-pair-major: qt[b, p, t2, j, c] = q[b, c, n] with
    # n = (2*t2 + j)*128 + p, matching the DoubleRow lhsT pair layout
    x = Z1.reshape(B, C, NT, P).astype(NP_FP8)
    return np.ascontiguousarray(x.transpose(0, 3, 2, 1)).reshape(
        B, P, NPAIR, 2, C)


def kernel(Z1, Zr, beta):
    Z1 = np.asarray(Z1, dtype=np.float32)
    Zr = np.asarray(Zr, dtype=np.float32)
    beta = np.asarray(beta, dtype=np.float32).reshape(1)

    qta = pack_qt(Z1)
    zrk = np.ascontiguousarray(Zr.reshape(B, C, N)).astype(NP_FP8)

    in_maps = []
    for i in range(NCORES):
        s = slice(i * BL, (i + 1) * BL)
        in_maps.append({"qt": qta[s], "zr": zrk[s], "beta": beta})

    nc = _get_program()
    res = run_bass_kernel_spmd(nc, in_maps, list(range(NCORES)))
    delta = np.concatenate(
        [np.asarray(r["out"]).astype(np.float32) for r in res.results], axis=0)
    return (Zr.reshape(B, C, N) + delta).reshape(B, C, H, W)


# revision 11
# speedup vs baseline: 2.1362x; 1.0754x over previous
"""Trainium2 Bass kernel for nn_Cross_Attention (B=16, C=256, H=W=96).

reference:
    q = Z1.reshape(B, C, N); k = Zr.reshape(B, C, N)         # N = H*W
    energy    = q @ k^T                                       # [B, C, C]
    attention = softmax(rowmax(energy) - energy, axis=-1)
    out       = attention @ k                                 # [B, C, N]
    return beta * out + Zr

Strategy: data-parallel over batch, 2 batches per NeuronCore on 8 cores.
All device I/O is fp8e4m3 and all matmuls run in fp8 with DoubleRow perf
mode (one PE instruction contracts a 256-deep pair of k-tiles), which cuts
both the HBM traffic and the PE time ~2x vs a bf16 formulation:
  - q^T is host-packed fp8 [P, 36, 2, C] (contraction-pair-major) so the
    energy matmul streams straight from DRAM with no on-chip transposes.
  - k  is the fp8 downcast of Zr, loaded once [C, N]; the energy matmul's
    k^T pair-tiles are derived on-chip with PE transpose-mode matmuls
    (fp8 transposes must write PSUM with element step 2 - walrus rule -
    so the psum tr tiles carry a trailing pad dim and the psum->SBUF
    repack copies read strided).
  - softmax(max - e) == exp(min - e) / sum(exp(min - e)) row-wise: only a
    row-min is needed, exp args are <= 0 (no overflow), sum >= 1.
  - beta and 1/sum are folded into the attention weights BEFORE the second
    matmul, so the device emits delta := beta * (attn @ k) in fp8 and the
    host adds the f32 residual:  out = Zr + delta.  When beta == 0 the
    folded weights are exactly zero, delta is exactly zero, and the
    returned output is bitwise Zr.
The psum->SBUF repack copies (k^T tiles and the delta downcast) are load
balanced across ScalarE / VectorE / GpSimdE so no single engine exceeds
the ~39us/core DMA roofline (14.2 MB of fp8 traffic at 360 GB/s).
"""

from contextlib import ExitStack

import ml_dtypes
import numpy as np

import concourse.bass as bass
import concourse.tile as tile
from concourse import bacc, mybir
from concourse.bass_utils import run_bass_kernel_spmd
from concourse.masks import make_identity

B, C, H, W = 16, 256, 96, 96
N = H * W                    # 9216
P = 128
NCORES = 8
BL = B // NCORES             # 2 batches per core
CT = C // P                  # 2 c-tiles of 128
NT = N // P                  # 72 contraction tiles
NPAIR = NT // 2              # 36 DoubleRow contraction pairs
QCH = 12                     # qt pairs per DMA chunk -> 3 chunks
GP = 2                       # pairs per transpose/repack group
NG = NPAIR // GP             # 18 groups per batch
KCC = 4                      # kb column chunks per c-tile row
KCW = N // KCC               # 2304 cols per kb chunk
OW = 512                     # mm2 psum chunk width
OPC = 2                      # mm2 psum chunks per outp tile -> copies of 1024
DCW = OPC * OW               # 1024: delta repack width
NDC = N // DCW               # 9 delta repacks per c-tile row
SCW = 3 * DCW                # 3072: store width (3 repacks per store)

F32 = mybir.dt.float32
U16 = mybir.dt.uint16
FP8 = mybir.dt.float8e4
NP_FP8 = ml_dtypes.float8_e4m3
DR = mybir.MatmulPerfMode.DoubleRow


class _CopyBalancer:
    """Round-robin psum->SBUF repack copies across ACT/DVE/Pool by
    projected busy-ns so no engine becomes the bottleneck."""

    def __init__(self, nc):
        # (issue fn, ns/elem plain, ns/elem 2x-capable, fixed ns).  GpSimd
        # is excluded: it cannot access PSUM (BIR verifier rule).  Only the
        # DVE has the 2x_1p fast path (2-byte packed operands).
        self.engines = [
            [nc.scalar.copy, 0.833, 0.833, 250.0, 0.0],
            [lambda out, in_: nc.vector.tensor_copy(out=out, in_=in_),
             1.042, 0.521, 215.0, 0.0],
        ]

    def charge(self, idx, ns):
        self.engines[idx][4] += ns

    def copy(self, out, in_, free, twox=False):
        r = 2 if twox else 1
        best = min(self.engines, key=lambda e: e[4] + free * e[r] + e[3])
        best[4] += free * best[r] + best[3]
        if best is self.engines[0]:
            best[0](out=out, in_=in_)
        else:
            best[0](out, in_)


def _build_program():
    nc = bacc.Bacc("TRN2", target_bir_lowering=False, debug=False,
                   num_devices=NCORES)

    qt_ext = nc.dram_tensor("qt", [BL, P, NPAIR, 2, C], FP8,
                            kind="ExternalInput")
    zr_ext = nc.dram_tensor("zr", [BL, C, N], FP8, kind="ExternalInput")
    beta_ext = nc.dram_tensor("beta", [1], F32, kind="ExternalInput")
    out_ext = nc.dram_tensor("out", [BL, C, N], FP8, kind="ExternalOutput")

    with tile.TileContext(nc) as tc, ExitStack() as ctx:
        kbp = ctx.enter_context(tc.tile_pool(name="kbp", bufs=2))
        qtp = ctx.enter_context(tc.tile_pool(name="qtp", bufs=6))
        kttp = ctx.enter_context(tc.tile_pool(name="kttp", bufs=38))
        expp = ctx.enter_context(tc.tile_pool(name="expp", bufs=2))
        attp = ctx.enter_context(tc.tile_pool(name="attp", bufs=2))
        atTp = ctx.enter_context(tc.tile_pool(name="atTp", bufs=2))
        deltap = ctx.enter_context(tc.tile_pool(name="deltap", bufs=4))
        statp = ctx.enter_context(tc.tile_pool(name="statp", bufs=8))
        singles = ctx.enter_context(tc.tile_pool(name="singles", bufs=1))
        engp = ctx.enter_context(tc.tile_pool(name="engp", bufs=1, space="PSUM"))
        trp = ctx.enter_context(tc.tile_pool(name="trp", bufs=3, space="PSUM"))
        outp = ctx.enter_context(tc.tile_pool(name="outp", bufs=2, space="PSUM"))

        cb = _CopyBalancer(nc)

        ident = singles.tile([P, P], FP8)
        make_identity(nc, ident)
        beta_sb = singles.tile([P, 1], F32)
        nc.gpsimd.dma_start(out=beta_sb, in_=beta_ext.ap().to_broadcast((P, 1)))

        # ---- emission helpers.  fp8 PE transposes must write PSUM with
        # element step 2 (walrus rule); the two n-tiles of a DoubleRow pair
        # interleave byte-wise into the same psum region so no space is
        # wasted, and the repack copy un-interleaves via a permuted AP ----
        def emit_tr(kb_b, g, ktts):
            trt = trp.tile([P, GP, 2, CT, P, 2], FP8, name="trt")
            for pr in range(GP):
                for j in range(2):
                    t = (GP * g + pr) * 2 + j
                    for dj in range(CT):
                        nc.tensor.transpose(
                            trt[:, pr, j, dj, :, 0],
                            kb_b[:, dj, t * P:(t + 1) * P],
                            ident)
            ktt = kttp.tile([P, GP, 2, CT * P], U16, name="ktt")
            cb.copy(ktt, trt.bitcast(U16), GP * 2 * CT * P, twox=True)
            ktts[g] = ktt.bitcast(FP8)

        def emit_emm(qt_b, g, eng, ktts):
            for pr in range(GP):
                t2 = GP * g + pr
                for ci in range(CT):
                    nc.tensor.matmul(
                        eng[ci],
                        lhsT=qt_b[t2 // QCH][:, t2 % QCH, :,
                                             ci * P:(ci + 1) * P],
                        rhs=ktts[g][:, pr, :, ::2],
                        start=(t2 == 0),
                        stop=(t2 == NPAIR - 1),
                        perf_mode=DR,
                    )

        def emit_softmax(eng):
            # softmax(max-e) = exp(min-e)/sum with beta/sum folded into the
            # fp8 attention weights; pair-transposed for the DR mm2
            atT = []
            for ci in range(CT):
                mn = statp.tile([P, 1], F32)
                nc.vector.tensor_reduce(out=mn, in_=eng[ci],
                                        axis=mybir.AxisListType.X,
                                        op=mybir.AluOpType.min)
                ex = expp.tile([P, C], F32)
                sm = statp.tile([P, 1], F32)
                nc.scalar.activation(out=ex, in_=eng[ci],
                                     func=mybir.ActivationFunctionType.Exp,
                                     bias=mn, scale=-1.0, accum_out=sm)
                rc = statp.tile([P, 1], F32)
                nc.vector.reciprocal(out=rc, in_=sm)
                rb = statp.tile([P, 1], F32)
                nc.vector.tensor_mul(out=rb, in0=rc, in1=beta_sb)
                at = attp.tile([P, C], FP8)
                nc.vector.tensor_scalar_mul(out=at, in0=ex, scalar1=rb)
                atr = trp.tile([P, GP, 2, CT, P, 2], FP8, name="atr",
                               tag="trt")
                for dj in range(CT):
                    nc.tensor.transpose(atr[:, 0, 0, dj, :, 0],
                                        at[:, dj * P:(dj + 1) * P], ident)
                att = atTp.tile([P, CT, P], FP8, name="atT")
                cb.copy(att, atr[:, 0, 0, :, :, 0], CT * P)
                atT.append(att)
            cb.charge(0, 1600)   # exp x2 on ACT
            cb.charge(1, 3000)   # softmax smalls on DVE
            return atT

        def emit_mm2_chunk(b, kb_b, atT, deltas, ci, off, width, borrow):
            # one slice of delta = attn_scaled @ k: DR matmuls with the full
            # 256-deep contraction per 512 of width, fp8 repack, store.  In
            # the batch-1 tail the transpose psum banks are dead, so every
            # other chunk borrows one (borrow=True) for a ~ring-5 pipeline
            if borrow:
                ps = trp.tile([P, width // OW, OW], F32, name="pst",
                              tag="trt")
            else:
                ps = outp.tile([P, width // OW, OW], F32, name="ps",
                               tag="ps")
            for q in range(width // OW):
                w0 = off + q * OW
                nc.tensor.matmul(
                    ps[:, q, :],
                    lhsT=atT[ci],
                    rhs=kb_b[:, :, w0:w0 + OW],
                    start=True, stop=True,
                    perf_mode=DR,
                )
            cb.copy(deltas[ci][:, off:off + width], ps, width)

        # ---- all loads up front on the sync queue: the DMA device order is
        # loads(b0), loads(b1), stores(b0), stores(b1), so stores never block
        # a load and the 360 GB/s stream stays saturated ----
        kb = [kbp.tile([P, CT, N], FP8, name="kb") for _ in range(BL)]
        qt = [[qtp.tile([P, QCH, 2, C], FP8, name="qt_t")
               for _ in range(NPAIR // QCH)] for _ in range(BL)]
        for b in range(BL):
            for cc in range(KCC):
                for cj in range(CT):
                    nc.sync.dma_start(
                        out=kb[b][:, cj, cc * KCW:(cc + 1) * KCW],
                        in_=zr_ext[b, cj * P:(cj + 1) * P,
                                   cc * KCW:(cc + 1) * KCW],
                    )
        for b in range(BL):
            for cc in range(NPAIR // QCH):
                nc.sync.dma_start(
                    out=qt[b][cc],
                    in_=qt_ext[b, :, cc * QCH:(cc + 1) * QCH, :, :])

        # ---- two-batch software pipeline.  Batch 0's energy streams behind
        # its loads; then batch 0's mm2 chunks interleave 1:1 with batch 1's
        # energy groups so the PE queue never head-of-line blocks and the
        # ACT/DVE repack streams stay packed; batch 1's mm2 is the tail ----
        engsl = [engp.tile([P, CT, C], F32, name="eng") for _ in range(BL)]
        eng = [[engsl[b][:, ci, :] for ci in range(CT)] for b in range(BL)]
        ktts = [[None] * NG for _ in range(BL)]
        deltas = [[deltap.tile([P, N], FP8, name="delta") for _ in range(CT)]
                  for _ in range(BL)]
        chunks = [(ci, w2 * DCW, DCW) for ci in range(CT)
                  for w2 in range(NDC)]

        for b in range(BL):
            for g in range(NG):
                emit_tr(kb[b], g, ktts[b])
        for g in range(NG):
            emit_emm(qt[0], g, eng[0], ktts[0])
        atT0 = emit_softmax(eng[0])

        for i in range(NG):
            ci, off, width = chunks[i]
            emit_mm2_chunk(0, kb[0], atT0, deltas[0], ci, off, width, False)
            emit_emm(qt[1], i, eng[1], ktts[1])
            if (off + width) % SCW == 0:
                nc.sync.dma_start(
                    out=out_ext[0, ci * P:(ci + 1) * P,
                                off + width - SCW:off + width],
                    in_=deltas[0][ci][:, off + width - SCW:off + width],
                )
        atT1 = emit_softmax(eng[1])

        # batch-1 tail: alternate 1024-wide outp chunks with 512-wide
        # borrowed-transpose-bank chunks for a deeper psum ring
        for ci in range(CT):
            for s in range(N // 1536):
                emit_mm2_chunk(1, kb[1], atT1, deltas[1], ci,
                               s * 1536, 1024, False)
                emit_mm2_chunk(1, kb[1], atT1, deltas[1], ci,
                               s * 1536 + 1024, 512, True)
                nc.sync.dma_start(
                    out=out_ext[1, ci * P:(ci + 1) * P,
                                s * 1536:(s + 1) * 1536],
                    in_=deltas[1][ci][:, s * 1536:(s + 1) * 1536],
                )

    nc.compile()
    return nc


_NC_CACHE = None


def _get_program():
    global _NC_CACHE
    if _NC_CACHE is None:
        _NC_CACHE = _build_program()
    return _NC_CACHE


def pack_qt(Z1):
    # fp8 q^T, contraction-pair-major: qt[b, p, t2, j, c] = q[b, c, n] with
    # n = (2*t2 + j)*128 + p, matching the DoubleRow lhsT pair layout
    x = Z1.reshape(B, C, NT, P).astype(NP_FP8)
    return np.ascontiguousarray(x.transpose(0, 3, 2, 1)).reshape(
        B, P, NPAIR, 2, C)


def kernel(Z1, Zr, beta):
    Z1 = np.asarray(Z1, dtype=np.float32)
    Zr = np.asarray(Zr, dtype=np.float32)
    beta = np.asarray(beta, dtype=np.float32).reshape(1)

    qta = pack_qt(Z1)
    zrk = np.ascontiguousarray(Zr.reshape(B, C, N)).astype(NP_FP8)

    in_maps = []
    for i in range(NCORES):
        s = slice(i * BL, (i + 1) * BL)
        in_maps.append({"qt": qta[s], "zr": zrk[s], "beta": beta})

    nc = _get_program()
    res = run_bass_kernel_spmd(nc, in_maps, list(range(NCORES)))
    delta = np.concatenate(
        [np.asarray(r["out"]).astype(np.float32) for r in res.results], axis=0)
    return (Zr.reshape(B, C, N) + delta).reshape(B, C, H, W)


# revision 12
# speedup vs baseline: 2.2481x; 1.0524x over previous
"""Trainium2 Bass kernel for nn_Cross_Attention (B=16, C=256, H=W=96).

reference:
    q = Z1.reshape(B, C, N); k = Zr.reshape(B, C, N)         # N = H*W
    energy    = q @ k^T                                       # [B, C, C]
    attention = softmax(rowmax(energy) - energy, axis=-1)
    out       = attention @ k                                 # [B, C, N]
    return beta * out + Zr

Strategy: data-parallel over batch, 2 batches per NeuronCore on 8 cores.
All device I/O is fp8e4m3 and all matmuls run in fp8 with DoubleRow perf
mode (one PE instruction contracts a 256-deep pair of k-tiles), which cuts
both the HBM traffic and the PE time ~2x vs a bf16 formulation:
  - q^T is host-packed fp8 [P, 36, 2, C] (contraction-pair-major) so the
    energy matmul streams straight from DRAM with no on-chip transposes.
  - k  is the fp8 downcast of Zr, loaded once [C, N]; the energy matmul's
    k^T pair-tiles are derived on-chip with PE transpose-mode matmuls
    (fp8 transposes must write PSUM with element step 2 - walrus rule -
    so the psum tr tiles carry a trailing pad dim and the psum->SBUF
    repack copies read strided).
  - softmax(max - e) == exp(min - e) / sum(exp(min - e)) row-wise: only a
    row-min is needed, exp args are <= 0 (no overflow), sum >= 1.
  - beta and 1/sum are folded into the attention weights BEFORE the second
    matmul, so the device emits delta := beta * (attn @ k) in fp8 and the
    host adds the f32 residual:  out = Zr + delta.  When beta == 0 the
    folded weights are exactly zero, delta is exactly zero, and the
    returned output is bitwise Zr.
The psum->SBUF repack copies (k^T tiles and the delta downcast) are load
balanced across ScalarE / VectorE / GpSimdE so no single engine exceeds
the ~39us/core DMA roofline (14.2 MB of fp8 traffic at 360 GB/s).
"""

from contextlib import ExitStack

import ml_dtypes
import numpy as np

import concourse.bass as bass
import concourse.tile as tile
from concourse import bacc, mybir
from concourse.bass_utils import run_bass_kernel_spmd
from concourse.masks import make_identity

B, C, H, W = 16, 256, 96, 96
N = H * W                    # 9216
P = 128
NCORES = 8
BL = B // NCORES             # 2 batches per core
CT = C // P                  # 2 c-tiles of 128
NT = N // P                  # 72 contraction tiles
NPAIR = NT // 2              # 36 DoubleRow contraction pairs
QCH = 12                     # qt pairs per DMA chunk -> 3 chunks
GP = 2                       # pairs per transpose/repack group
NG = NPAIR // GP             # 18 groups per batch
KCC = 4                      # kb column chunks per c-tile row
KCW = N // KCC               # 2304 cols per kb chunk
OW = 512                     # mm2 psum chunk width
OPC = 2                      # mm2 psum chunks per outp tile -> copies of 1024
DCW = OPC * OW               # 1024: delta repack width
NDC = N // DCW               # 9 delta repacks per c-tile row
SCW = 3 * DCW                # 3072: store width (3 repacks per store)

F32 = mybir.dt.float32
U16 = mybir.dt.uint16
FP8 = mybir.dt.float8e4
NP_FP8 = ml_dtypes.float8_e4m3
DR = mybir.MatmulPerfMode.DoubleRow


class _CopyBalancer:
    """Round-robin psum->SBUF repack copies across ACT/DVE/Pool by
    projected busy-ns so no engine becomes the bottleneck."""

    def __init__(self, nc):
        # (issue fn, ns/elem plain, ns/elem 2x-capable, fixed ns).  GpSimd
        # is excluded: it cannot access PSUM (BIR verifier rule).  Only the
        # DVE has the 2x_1p fast path (2-byte packed operands).
        self.engines = [
            [nc.scalar.copy, 0.833, 0.833, 250.0, 0.0],
            [lambda out, in_: nc.vector.tensor_copy(out=out, in_=in_),
             1.042, 0.521, 215.0, 0.0],
        ]

    def charge(self, idx, ns):
        self.engines[idx][4] += ns

    def copy(self, out, in_, free, twox=False):
        r = 2 if twox else 1
        best = min(self.engines, key=lambda e: e[4] + free * e[r] + e[3])
        best[4] += free * best[r] + best[3]
        if best is self.engines[0]:
            best[0](out=out, in_=in_)
        else:
            best[0](out, in_)


def _build_program():
    nc = bacc.Bacc("TRN2", target_bir_lowering=False, debug=False,
                   num_devices=NCORES)

    qt_ext = nc.dram_tensor("qt", [BL, P, NPAIR, 2, C], FP8,
                            kind="ExternalInput")
    zr_ext = nc.dram_tensor("zr", [BL, C, N], FP8, kind="ExternalInput")
    beta_ext = nc.dram_tensor("beta", [1], F32, kind="ExternalInput")
    out_ext = nc.dram_tensor("out", [BL, C, N], FP8, kind="ExternalOutput")

    with tile.TileContext(nc) as tc, ExitStack() as ctx:
        kbp = ctx.enter_context(tc.tile_pool(name="kbp", bufs=2))
        qtp = ctx.enter_context(tc.tile_pool(name="qtp", bufs=6))
        kttp = ctx.enter_context(tc.tile_pool(name="kttp", bufs=38))
        expp = ctx.enter_context(tc.tile_pool(name="expp", bufs=2))
        attp = ctx.enter_context(tc.tile_pool(name="attp", bufs=2))
        atTp = ctx.enter_context(tc.tile_pool(name="atTp", bufs=2))
        deltap = ctx.enter_context(tc.tile_pool(name="deltap", bufs=4))
        statp = ctx.enter_context(tc.tile_pool(name="statp", bufs=8))
        singles = ctx.enter_context(tc.tile_pool(name="singles", bufs=1))
        engp = ctx.enter_context(tc.tile_pool(name="engp", bufs=1, space="PSUM"))
        trp = ctx.enter_context(tc.tile_pool(name="trp", bufs=3, space="PSUM"))
        outp = ctx.enter_context(tc.tile_pool(name="outp", bufs=2, space="PSUM"))

        cb = _CopyBalancer(nc)

        ident = singles.tile([P, P], FP8)
        make_identity(nc, ident)
        beta_sb = singles.tile([P, 1], F32)
        nc.gpsimd.dma_start(out=beta_sb, in_=beta_ext.ap().to_broadcast((P, 1)))

        # ---- emission helpers.  fp8 PE transposes must write PSUM with
        # element step 2 (walrus rule); the two n-tiles of a DoubleRow pair
        # interleave byte-wise into the same psum region so no space is
        # wasted, and the repack copy un-interleaves via a permuted AP ----
        def emit_tr(kb_b, g, ktts, borrow=False):
            # 3 trp banks + (early, while mm2 is idle) 2 borrowed outp
            # slots give the transpose stream a ~ring-5 psum pipeline
            if borrow:
                trt = trp.tile([P, GP, 2, CT, P, 2], FP8, name="trt",
                               tag="trt")
            else:
                trt = outp.tile([P, GP, 2, CT, P, 2], FP8, name="trto",
                                tag="ps")
            for pr in range(GP):
                for j in range(2):
                    t = (GP * g + pr) * 2 + j
                    for dj in range(CT):
                        nc.tensor.transpose(
                            trt[:, pr, j, dj, :, 0],
                            kb_b[:, dj, t * P:(t + 1) * P],
                            ident)
            ktt = kttp.tile([P, GP, 2, CT * P], U16, name="ktt")
            cb.copy(ktt, trt.bitcast(U16), GP * 2 * CT * P, twox=True)
            ktts[g] = ktt.bitcast(FP8)

        def emit_emm(qt_b, g, eng, ktts):
            for pr in range(GP):
                t2 = GP * g + pr
                for ci in range(CT):
                    nc.tensor.matmul(
                        eng[ci],
                        lhsT=qt_b[t2 // QCH][:, t2 % QCH, :,
                                             ci * P:(ci + 1) * P],
                        rhs=ktts[g][:, pr, :, ::2],
                        start=(t2 == 0),
                        stop=(t2 == NPAIR - 1),
                        perf_mode=DR,
                    )

        def emit_softmax(eng):
            # softmax(max-e) = exp(min-e)/sum with beta/sum folded into the
            # fp8 attention weights; pair-transposed for the DR mm2
            atT = []
            for ci in range(CT):
                mn = statp.tile([P, 1], F32)
                nc.vector.tensor_reduce(out=mn, in_=eng[ci],
                                        axis=mybir.AxisListType.X,
                                        op=mybir.AluOpType.min)
                ex = expp.tile([P, C], F32)
                sm = statp.tile([P, 1], F32)
                nc.scalar.activation(out=ex, in_=eng[ci],
                                     func=mybir.ActivationFunctionType.Exp,
                                     bias=mn, scale=-1.0, accum_out=sm)
                rc = statp.tile([P, 1], F32)
                nc.vector.reciprocal(out=rc, in_=sm)
                rb = statp.tile([P, 1], F32)
                nc.vector.tensor_mul(out=rb, in0=rc, in1=beta_sb)
                at = attp.tile([P, C], FP8)
                nc.vector.tensor_scalar_mul(out=at, in0=ex, scalar1=rb)
                atr = trp.tile([P, GP, 2, CT, P, 2], FP8, name="atr",
                               tag="trt")
                for dj in range(CT):
                    nc.tensor.transpose(atr[:, 0, 0, dj, :, 0],
                                        at[:, dj * P:(dj + 1) * P], ident)
                att = atTp.tile([P, CT, P], FP8, name="atT")
                cb.copy(att, atr[:, 0, 0, :, :, 0], CT * P)
                atT.append(att)
            cb.charge(0, 1600)   # exp x2 on ACT
            cb.charge(1, 3000)   # softmax smalls on DVE
            return atT

        def emit_mm2_chunk(b, kb_b, atT, deltas, ci, off, width, borrow):
            # one slice of delta = attn_scaled @ k: DR matmuls with the full
            # 256-deep contraction per 512 of width, fp8 repack, store.  In
            # the batch-1 tail the transpose psum banks are dead, so every
            # other chunk borrows one (borrow=True) for a ~ring-5 pipeline
            if borrow:
                ps = trp.tile([P, width // OW, OW], F32, name="pst",
                              tag="trt")
            else:
                ps = outp.tile([P, width // OW, OW], F32, name="ps",
                               tag="ps")
            for q in range(width // OW):
                w0 = off + q * OW
                nc.tensor.matmul(
                    ps[:, q, :],
                    lhsT=atT[ci],
                    rhs=kb_b[:, :, w0:w0 + OW],
                    start=True, stop=True,
                    perf_mode=DR,
                )
            cb.copy(deltas[ci][:, off:off + width], ps, width)

        # ---- all loads up front on the sync queue: the DMA device order is
        # loads(b0), loads(b1), stores(b0), stores(b1), so stores never block
        # a load and the 360 GB/s stream stays saturated ----
        kb = [kbp.tile([P, CT, N], FP8, name="kb") for _ in range(BL)]
        qt = [[qtp.tile([P, QCH, 2, C], FP8, name="qt_t")
               for _ in range(NPAIR // QCH)] for _ in range(BL)]
        for b in range(BL):
            for cc in range(KCC):
                for cj in range(CT):
                    nc.sync.dma_start(
                        out=kb[b][:, cj, cc * KCW:(cc + 1) * KCW],
                        in_=zr_ext[b, cj * P:(cj + 1) * P,
                                   cc * KCW:(cc + 1) * KCW],
                    )
        for b in range(BL):
            for cc in range(NPAIR // QCH):
                nc.sync.dma_start(
                    out=qt[b][cc],
                    in_=qt_ext[b, :, cc * QCH:(cc + 1) * QCH, :, :])

        # ---- two-batch software pipeline.  Batch 0's energy streams behind
        # its loads; then batch 0's mm2 chunks interleave 1:1 with batch 1's
        # energy groups so the PE queue never head-of-line blocks and the
        # ACT/DVE repack streams stay packed; batch 1's mm2 is the tail ----
        engsl = [engp.tile([P, CT, C], F32, name="eng") for _ in range(BL)]
        eng = [[engsl[b][:, ci, :] for ci in range(CT)] for b in range(BL)]
        ktts = [[None] * NG for _ in range(BL)]
        deltas = [[deltap.tile([P, N], FP8, name="delta") for _ in range(CT)]
                  for _ in range(BL)]
        chunks = [(ci, w2 * DCW, DCW) for ci in range(CT)
                  for w2 in range(NDC)]

        for b in range(BL):
            for g in range(NG):
                emit_tr(kb[b], g, ktts[b], borrow=(g % 5 < 3))
        for g in range(NG):
            emit_emm(qt[0], g, eng[0], ktts[0])
        atT0 = emit_softmax(eng[0])

        for i in range(NG):
            ci, off, width = chunks[i]
            emit_mm2_chunk(0, kb[0], atT0, deltas[0], ci, off, width, False)
            emit_emm(qt[1], i, eng[1], ktts[1])
            if (off + width) % SCW == 0:
                nc.sync.dma_start(
                    out=out_ext[0, ci * P:(ci + 1) * P,
                                off + width - SCW:off + width],
                    in_=deltas[0][ci][:, off + width - SCW:off + width],
                )
        atT1 = emit_softmax(eng[1])

        # batch-1 tail: alternate 1024-wide outp chunks with 512-wide
        # borrowed-transpose-bank chunks for a deeper psum ring
        for ci in range(CT):
            for s in range(N // 1536):
                emit_mm2_chunk(1, kb[1], atT1, deltas[1], ci,
                               s * 1536, 1024, False)
                emit_mm2_chunk(1, kb[1], atT1, deltas[1], ci,
                               s * 1536 + 1024, 512, True)
                nc.sync.dma_start(
                    out=out_ext[1, ci * P:(ci + 1) * P,
                                s * 1536:(s + 1) * 1536],
                    in_=deltas[1][ci][:, s * 1536:(s + 1) * 1536],
                )

    nc.compile()
    return nc


_NC_CACHE = None


def _get_program():
    global _NC_CACHE
    if _NC_CACHE is None:
        _NC_CACHE = _build_program()
    return _NC_CACHE


def pack_qt(Z1):
    # fp8 q^T, contraction-pair-major: qt[b, p, t2, j, c] = q[b, c, n] with
    # n = (2*t2 + j)*128 + p, matching the DoubleRow lhsT pair layout
    x = Z1.reshape(B, C, NT, P).astype(NP_FP8)
    return np.ascontiguousarray(x.transpose(0, 3, 2, 1)).reshape(
        B, P, NPAIR, 2, C)


def kernel(Z1, Zr, beta):
    Z1 = np.asarray(Z1, dtype=np.float32)
    Zr = np.asarray(Zr, dtype=np.float32)
    beta = np.asarray(beta, dtype=np.float32).reshape(1)

    qta = pack_qt(Z1)
    zrk = np.ascontiguousarray(Zr.reshape(B, C, N)).astype(NP_FP8)

    in_maps = []
    for i in range(NCORES):
        s = slice(i * BL, (i + 1) * BL)
        in_maps.append({"qt": qta[s], "zr": zrk[s], "beta": beta})

    nc = _get_program()
    res = run_bass_kernel_spmd(nc, in_maps, list(range(NCORES)))
    delta = np.concatenate(
        [np.asarray(r["out"]).astype(np.float32) for r in res.results], axis=0)
    return (Zr.reshape(B, C, N) + delta).reshape(B, C, H, W)
